# revision 1
# baseline (speedup 1.0000x reference)
"""Trainium2 Bass kernel for nn_CrossAttention_59717225284223.

Full-input contract: kernel(**inputs) takes the complete [4,256,8192] tensors,
shards across 8 NeuronCores internally (core i -> batch i//2, N-half i%2; the
x2/KV side is recomputed per batch pair so no collectives are needed), and
returns the full [4,256,8192] float32 output.

v2: bf16 matmuls (FWL weight loads), bias/+1 terms folded into rank-1 matmul
updates, LN2 mean folded into centered W2, activations on the Scalar engine
(single act table: exp/ln/relu/copy/square), elementwise work spread across
Vector/GpSimd/Scalar so no single engine saturates.
"""

import os
import sys

import numpy as np

for _p in (
    "/root/.axon_site",
    "/root/.axon_site/_ro/trn_rl_repo",
    "/opt/trn_rl_repo",
):
    if os.path.isdir(_p) and _p not in sys.path:
        sys.path.append(_p)

import concourse.bass as bass  # noqa: E402
import concourse.tile as tile  # noqa: E402
from concourse import bacc, mybir  # noqa: E402
from concourse.bass_utils import run_bass_kernel_spmd  # noqa: E402

F32 = mybir.dt.float32
BF16 = mybir.dt.bfloat16
OP = mybir.AluOpType
AF = mybir.ActivationFunctionType

B, C, N = 4, 256, 8192
H, D = 4, 64
L = N // 2          # positions per core
NT = N // 128       # x2-side 128-position tiles
NCH = L // 512      # 512-position chunks per core
LN_EPS = 1e-5
BN_EPS = 1e-5
ATTN_EPS = 1e-6

_CACHE = {}


def _build():
    nc = bacc.Bacc(None, target_bir_lowering=False)

    x1 = nc.dram_tensor("x1", [C, L], BF16, kind="ExternalInput")
    x2 = nc.dram_tensor("x2", [C, N], BF16, kind="ExternalInput")
    wkv = nc.dram_tensor("wkv", [C, 2 * C], BF16, kind="ExternalInput")
    wq = nc.dram_tensor("wq", [C, C], BF16, kind="ExternalInput")
    wa = nc.dram_tensor("wa", [C, C], BF16, kind="ExternalInput")
    w1a = nc.dram_tensor("w1a", [C, 2 * C], BF16, kind="ExternalInput")
    w1b = nc.dram_tensor("w1b", [C, 2 * C], BF16, kind="ExternalInput")
    w2 = nc.dram_tensor("w2", [2 * C, C], BF16, kind="ExternalInput")
    bkr = nc.dram_tensor("bkr", [1, C], BF16, kind="ExternalInput")
    bvr = nc.dram_tensor("bvr", [1, C], F32, kind="ExternalInput")
    bqn = nc.dram_tensor("bqn", [C, 1], F32, kind="ExternalInput")
    bq1 = nc.dram_tensor("bq1", [C, 1], F32, kind="ExternalInput")
    ba = nc.dram_tensor("ba", [C, 1], F32, kind="ExternalInput")
    hbv = nc.dram_tensor("hb", [2 * C, 1], F32, kind="ExternalInput")
    g2 = nc.dram_tensor("g2", [C, 1], F32, kind="ExternalInput")
    thr = nc.dram_tensor("thr", [1, 1], F32, kind="ExternalInput")
    ident = nc.dram_tensor("ident", [128, 128], BF16, kind="ExternalInput")
    out = nc.dram_tensor("out", [C, L], F32, kind="ExternalOutput")

    x1r = x1[:, :].rearrange("(t p) n -> p t n", p=128)
    x2r = x2[:, :].rearrange("(t p) n -> p t n", p=128)
    outr = out[:, :].rearrange("(t p) n -> p t n", p=128)

    def bcast(ap, p):
        return bass.AP(tensor=ap.tensor, offset=ap.offset,
                       ap=[[0, p]] + [list(d) for d in ap.ap[1:]])

    with tile.TileContext(nc) as tc:
        with tc.tile_pool(name="consts", bufs=1) as consts, \
             tc.tile_pool(name="resident", bufs=1) as res:
            # ---- constants ----
            wkv_sb = consts.tile([128, 2, 2 * C], BF16)
            nc.sync.dma_start(out=wkv_sb, in_=wkv[:, :].rearrange(
                "(t p) o -> p t o", p=128))
            wq_sb = consts.tile([128, 2, C], BF16)
            nc.sync.dma_start(out=wq_sb, in_=wq[:, :].rearrange(
                "(t p) o -> p t o", p=128))
            wa_sb = consts.tile([128, 2, C], BF16)
            nc.sync.dma_start(out=wa_sb, in_=wa[:, :].rearrange(
                "(t p) o -> p t o", p=128))
            w1a_sb = consts.tile([128, 2, 2 * C], BF16)
            nc.sync.dma_start(out=w1a_sb, in_=w1a[:, :].rearrange(
                "(t p) o -> p t o", p=128))
            w1b_sb = consts.tile([128, 2, 2 * C], BF16)
            nc.sync.dma_start(out=w1b_sb, in_=w1b[:, :].rearrange(
                "(t p) o -> p t o", p=128))
            w2_sb = consts.tile([128, 4, C], BF16)
            nc.sync.dma_start(out=w2_sb, in_=w2[:, :].rearrange(
                "(t p) o -> p t o", p=128))
            ident_sb = consts.tile([128, 128], BF16)
            nc.sync.dma_start(out=ident_sb, in_=ident[:, :])
            bkr_sb = consts.tile([1, C], BF16)
            nc.sync.dma_start(out=bkr_sb, in_=bkr[:, :])
            bvr_sb = consts.tile([1, C], F32)
            nc.sync.dma_start(out=bvr_sb, in_=bvr[:, :])

            bqn_sb = consts.tile([128, 2], F32)
            bq1_sb = consts.tile([128, 2], F32)
            ba_sb = consts.tile([128, 2], F32)
            g2_sb = consts.tile([128, 2], F32)
            for t in range(2):
                sl = slice(t * 128, (t + 1) * 128)
                nc.sync.dma_start(out=bqn_sb[:, t:t + 1], in_=bqn[sl, :])
                nc.sync.dma_start(out=bq1_sb[:, t:t + 1], in_=bq1[sl, :])
                nc.sync.dma_start(out=ba_sb[:, t:t + 1], in_=ba[sl, :])
                nc.sync.dma_start(out=g2_sb[:, t:t + 1], in_=g2[sl, :])
            hb_sb = consts.tile([128, 4], F32)
            for t in range(4):
                nc.sync.dma_start(out=hb_sb[:, t:t + 1],
                                  in_=hbv[t * 128:(t + 1) * 128, :])
            thr_bc = consts.tile([128, 1], F32)
            nc.gpsimd.dma_start(out=thr_bc, in_=bcast(thr[:, :], 128))
            ones_r = consts.tile([1, 128], BF16)
            nc.gpsimd.memset(ones_r, 1.0)
            ones_c = consts.tile([128, 1], BF16)
            nc.gpsimd.memset(ones_c, 1.0)
            lneps = consts.tile([128, 1], F32)
            nc.vector.memset(lneps, LN_EPS)
            eps11 = consts.tile([1, 1], F32)
            nc.vector.memset(eps11, LN_EPS)

            # ---- resident activations ----
            x1_sb = res.tile([128, 2, L], BF16)
            q_sb = res.tile([128, 2, L], BF16)
            msgn_sb = res.tile([128, 2, L], BF16)
            kvbd = res.tile([128, 2, 260], BF16)
            esum_sb = res.tile([128, 2], F32)

            # ================= phase 1: x2 side (full N) =================
            with tc.tile_pool(name="x2p", bufs=3) as x2p, \
                 tc.tile_pool(name="kvbp", bufs=6) as kvbp, \
                 tc.tile_pool(name="sc1", bufs=6) as sc1, \
                 tc.tile_pool(name="cps", bufs=3, space="PSUM") as cps, \
                 tc.tile_pool(name="kvps", bufs=1, space="PSUM") as kvps:
                kv_ps = [kvps.tile([128, 258], F32, name=f"kv_ps{m}",
                                   tag=f"kv{m}") for m in range(2)]
                for ch in range(N // 512):
                    x2t = x2p.tile([128, 2, 512], BF16)
                    nc.sync.dma_start(
                        out=x2t, in_=x2r[:, :, ch * 512:(ch + 1) * 512])
                    for s in range(4):
                        i = ch * 4 + s
                        cp = cps.tile([128, 2 * C], F32)
                        nc.tensor.matmul(cp, x2t[:, 0, s * 128:(s + 1) * 128],
                                         wkv_sb[:, 0, :], start=True, stop=False)
                        nc.tensor.matmul(cp[:, 0:C], ones_r, bkr_sb,
                                         start=False, stop=False)
                        nc.tensor.matmul(cp, x2t[:, 1, s * 128:(s + 1) * 128],
                                         wkv_sb[:, 1, :], start=False, stop=True)
                        kvb = kvbp.tile([128, 514], BF16)
                        # elu(k)+1 = max(k+1, exp(min(k,0))); min via -relu(-k)
                        rn = sc1.tile([128, C], F32, name="rn", tag="rn")
                        nc.scalar.activation(out=rn, in_=cp[:, 0:C],
                                             func=AF.Relu, scale=-1.0)
                        ex = sc1.tile([128, C], F32, name="ex", tag="ex")
                        nc.scalar.activation(out=ex, in_=rn, func=AF.Exp,
                                             scale=-1.0)
                        nc.vector.scalar_tensor_tensor(
                            out=kvb[:, 0:C], in0=cp[:, 0:C], scalar=1.0,
                            in1=ex, op0=OP.add, op1=OP.max)
                        nc.vector.tensor_copy(out=kvb[:, C:2 * C],
                                              in_=cp[:, C:2 * C])
                        nc.gpsimd.memset(kvb[:, 512:514], 1.0)
                        nc.tensor.matmul(kv_ps[0], kvb[:, 0:128],
                                         kvb[:, 256:514],
                                         start=(i == 0), stop=(i == NT - 1))
                        nc.tensor.matmul(kv_ps[1], kvb[:, 128:256],
                                         kvb[:, 256:514],
                                         start=(i == 0), stop=(i == NT - 1))

                # ---- KV fixup: V-bias rank-1 term ----
                # kvb K-half already holds K=elu(k)+1, so
                # KVfix = KVps + Ksum (x) bv, with Ksum = KVps[:, 256].
                bv_bc = sc1.tile([128, C], F32, name="bvbc", tag="bvbc")
                nc.gpsimd.partition_broadcast(bv_bc, bvr_sb)
                for t in range(2):
                    nc.vector.tensor_copy(out=esum_sb[:, t:t + 1],
                                          in_=kv_ps[t][:, 256:257])
                nc.gpsimd.memset(kvbd, 0.0)
                for t in range(2):
                    for hh in range(2):
                        h = t * 2 + hh
                        rsl = slice(hh * 64, hh * 64 + 64)
                        csl = slice(h * 64, h * 64 + 64)
                        nc.vector.scalar_tensor_tensor(
                            out=kvbd[rsl, t, csl], in0=bv_bc[rsl, csl],
                            scalar=esum_sb[rsl, t:t + 1],
                            in1=kv_ps[t][rsl, csl],
                            op0=OP.mult, op1=OP.add)
                        nc.gpsimd.tensor_copy(
                            out=kvbd[rsl, t, 256 + h:257 + h],
                            in_=esum_sb[rsl, t:t + 1])

            # ============ phase 2+3: q conv, msg, LN1 ============
            with tc.tile_pool(name="sc2", bufs=3) as sc2, \
                 tc.tile_pool(name="sc3", bufs=5) as sc3, \
                 tc.tile_pool(name="stat", bufs=4) as stat, \
                 tc.tile_pool(name="qaps", bufs=2, space="PSUM") as qaps, \
                 tc.tile_pool(name="msgps", bufs=1, space="PSUM") as msgps, \
                 tc.tile_pool(name="trps", bufs=2, space="PSUM") as trps:
                for ch in range(NCH):
                    sl = slice(ch * 512, (ch + 1) * 512)
                    nc.sync.dma_start(out=x1_sb[:, :, sl], in_=x1r[:, :, sl])
                    for m in range(2):
                        qp = qaps.tile([128, 512], F32)
                        nc.tensor.matmul(qp, wq_sb[:, 0, m * 128:(m + 1) * 128],
                                         x1_sb[:, 0, sl], start=True, stop=False)
                        nc.tensor.matmul(qp, wq_sb[:, 1, m * 128:(m + 1) * 128],
                                         x1_sb[:, 1, sl], start=False, stop=True)
                        rq = sc2.tile([128, 512], F32, name="rq", tag="rq")
                        nc.scalar.activation(out=rq, in_=qp, func=AF.Relu,
                                             scale=-1.0, bias=bqn_sb[:, m:m + 1])
                        exq = sc2.tile([128, 512], F32, name="exq", tag="exq")
                        nc.scalar.activation(out=exq, in_=rq, func=AF.Exp,
                                             scale=-1.0)
                        nc.vector.scalar_tensor_tensor(
                            out=q_sb[:, m, sl], in0=qp,
                            scalar=bq1_sb[:, m:m + 1], in1=exq,
                            op0=OP.add, op1=OP.max)
                    zsc = stat.tile([128, 16], F32, name="zsc", tag="zsc")
                    mps = []
                    for s_ in range(4):
                        l0 = ch * 512 + s_ * 128
                        lsl = slice(l0, l0 + 128)
                        mp = msgps.tile([128, 260], F32, name=f"mp{s_}",
                                        tag=f"mp{s_}")
                        nc.tensor.matmul(mp, q_sb[:, 0, lsl], kvbd[:, 0, :],
                                         start=True, stop=False)
                        nc.tensor.matmul(mp, q_sb[:, 1, lsl], kvbd[:, 1, :],
                                         start=False, stop=True)
                        nc.vector.tensor_copy(out=zsc[:, s_ * 4:s_ * 4 + 4],
                                              in_=mp[:, 256:260])
                        mps.append(mp)
                    mk = stat.tile([128, 16], F32, name="mk", tag="mk")
                    nc.vector.tensor_scalar(
                        out=mk, in0=zsc, scalar1=thr_bc,
                        scalar2=None, op0=OP.is_gt)
                    nc.vector.tensor_tensor(out=mk, in0=zsc, in1=mk,
                                            op=OP.mult)
                    nc.vector.tensor_scalar(
                        out=mk, in0=mk, scalar1=ATTN_EPS, scalar2=None,
                        op0=OP.add)
                    zt = stat.tile([128, 16], F32, name="zt", tag="zt")
                    nc.vector.reciprocal_approx_fast(out=zt, in_=mk)
                    mv8 = stat.tile([128, 4, 2], F32, name="mv8", tag="mv8")
                    mss = []
                    for s_ in range(4):
                        mp = mps[s_]
                        ms = sc3.tile([128, C], F32, name="ms", tag="ms")
                        zb = zt[:, s_ * 4:s_ * 4 + 4]
                        zb = bass.AP(tensor=zb.tensor, offset=zb.offset,
                                     ap=[list(zb.ap[0]), list(zb.ap[1]),
                                         [0, 64]])
                        nc.vector.tensor_tensor(
                            out=ms.rearrange("p (h d) -> p h d", h=4),
                            in0=mp[:, 0:256].rearrange("p (h d) -> p h d", h=4),
                            in1=zb, op=OP.mult)
                        st6 = stat.tile([128, 6], F32, name="st6", tag="st6")
                        nc.vector.bn_stats(out=st6, in_=ms)
                        nc.vector.bn_aggr(out=mv8[:, s_, :], in_=st6)
                        mss.append(ms)
                    # rz = 1/sqrt(var+eps) = exp(-0.5*ln(var+eps))
                    lnv = stat.tile([128, 4], F32, name="lnv", tag="lnv")
                    nc.scalar.activation(out=lnv, in_=mv8[:, :, 1],
                                         func=AF.Ln, bias=lneps)
                    rz = stat.tile([128, 4], F32, name="rz", tag="rz")
                    nc.scalar.activation(out=rz, in_=lnv, func=AF.Exp,
                                         scale=-0.5)
                    for s_ in range(4):
                        l0 = ch * 512 + s_ * 128
                        lsl = slice(l0, l0 + 128)
                        msn = sc3.tile([128, C], BF16, name="msn", tag="msn")
                        eng = nc.vector if s_ % 2 == 0 else nc.gpsimd
                        eng.tensor_scalar(
                            out=msn, in0=mss[s_], scalar1=mv8[:, s_, 0:1],
                            scalar2=rz[:, s_:s_ + 1],
                            op0=OP.subtract, op1=OP.mult)
                        for t in range(2):
                            tp = trps.tile([128, 128], BF16)
                            nc.tensor.transpose(
                                tp, msn[:, t * 128:(t + 1) * 128], ident_sb)
                            if t == 0:
                                nc.scalar.activation(out=msgn_sb[:, t, lsl],
                                                     in_=tp, func=AF.Copy)
                            else:
                                nc.vector.tensor_copy(out=msgn_sb[:, t, lsl],
                                                      in_=tp)

            # ================= phase 4: MLP + LN2 + final =================
            with tc.tile_pool(name="hpool", bufs=5) as hpool, \
                 tc.tile_pool(name="sc4", bufs=4) as sc4, \
                 tc.tile_pool(name="bcp", bufs=2) as bcp, \
                 tc.tile_pool(name="outp", bufs=3) as outp, \
                 tc.tile_pool(name="hps", bufs=2, space="PSUM") as hps, \
                 tc.tile_pool(name="o2ps", bufs=1, space="PSUM") as o2ps, \
                 tc.tile_pool(name="stps", bufs=1, space="PSUM") as stps, \
                 tc.tile_pool(name="augps", bufs=2, space="PSUM") as augps:
                for ch in range(NCH):
                    sl = slice(ch * 512, (ch + 1) * 512)
                    hsb = []
                    for m in range(4):
                        mc = slice(m * 128, (m + 1) * 128)
                        hp = hps.tile([128, 512], F32)
                        nc.tensor.matmul(hp, w1a_sb[:, 0, mc], x1_sb[:, 0, sl],
                                         start=True, stop=False)
                        nc.tensor.matmul(hp, w1a_sb[:, 1, mc], x1_sb[:, 1, sl],
                                         start=False, stop=False)
                        nc.tensor.matmul(hp, w1b_sb[:, 0, mc],
                                         msgn_sb[:, 0, sl],
                                         start=False, stop=False)
                        nc.tensor.matmul(hp, w1b_sb[:, 1, mc],
                                         msgn_sb[:, 1, sl],
                                         start=False, stop=True)
                        ht = hpool.tile([128, 512], BF16)
                        if m % 2 == 0:
                            nc.scalar.activation(out=ht, in_=hp, func=AF.Relu,
                                                 bias=hb_sb[:, m:m + 1])
                        else:
                            nc.vector.tensor_scalar(
                                out=ht, in0=hp, scalar1=hb_sb[:, m:m + 1],
                                scalar2=0.0, op0=OP.add, op1=OP.max)
                        hsb.append(ht)
                    o2p = [o2ps.tile([128, 512], F32, name=f"o2p{m2}",
                                     tag=f"o2_{m2}") for m2 in range(2)]
                    for m2 in range(2):
                        mc2 = slice(m2 * 128, (m2 + 1) * 128)
                        for k in range(4):
                            nc.tensor.matmul(o2p[m2], w2_sb[:, k, mc2], hsb[k],
                                             start=(k == 0), stop=(k == 3))
                    # LN2: W2 pre-centered, so o2p is mean-free. var via
                    # sum of squares (Square on scalar, ones-matmul reduce).
                    sq = []
                    for m2 in range(2):
                        s_ = sc4.tile([128, 512], BF16, name=f"sq{m2}",
                                      tag=f"sq{m2}")
                        nc.scalar.activation(out=s_, in_=o2p[m2],
                                             func=AF.Square)
                        sq.append(s_)
                    ssq = stps.tile([1, 512], F32, tag="ssq")
                    nc.tensor.matmul(ssq, ones_c, sq[0], start=True, stop=False)
                    nc.tensor.matmul(ssq, ones_c, sq[1], start=False, stop=True)
                    # rstd = exp(-0.5*ln(ssq/C + eps))
                    lnv2 = sc4.tile([1, 512], F32, name="lnv2", tag="lnv2")
                    nc.scalar.activation(out=lnv2, in_=ssq, func=AF.Ln,
                                         scale=1.0 / C, bias=eps11)
                    rstd = sc4.tile([1, 512], F32, name="rstd", tag="rstd")
                    nc.scalar.activation(out=rstd, in_=lnv2, func=AF.Exp,
                                         scale=-0.5)
                    rstd_bc = bcp.tile([128, 512], F32, tag="rstd_bc")
                    nc.gpsimd.partition_broadcast(rstd_bc, rstd)
                    for m2 in range(2):
                        ap_ = augps.tile([128, 512], F32)
                        nc.tensor.matmul(ap_, wa_sb[:, 0, m2 * 128:(m2 + 1) * 128],
                                         x1_sb[:, 0, sl], start=True, stop=False)
                        nc.tensor.matmul(ap_, wa_sb[:, 1, m2 * 128:(m2 + 1) * 128],
                                         x1_sb[:, 1, sl], start=False, stop=True)
                        t1 = sc4.tile([128, 512], F32, name=f"t1{m2}",
                                      tag=f"t1{m2}")
                        nc.vector.scalar_tensor_tensor(
                            out=t1, in0=o2p[m2], scalar=g2_sb[:, m2:m2 + 1],
                            in1=rstd_bc, op0=OP.mult, op1=OP.mult)
                        ot = outp.tile([128, 512], F32)
                        nc.vector.scalar_tensor_tensor(
                            out=ot, in0=ap_, scalar=ba_sb[:, m2:m2 + 1], in1=t1,
                            op0=OP.add, op1=OP.add)
                        nc.sync.dma_start(out=outr[:, m2, sl], in_=ot)

    nc.compile()
    return nc


def _host_prep(inputs):
    """Fold BN/LN affine params into weights; build per-core input maps."""
    import ml_dtypes
    f32 = np.float32
    bf16 = ml_dtypes.bfloat16
    x1 = np.asarray(inputs["x1"], f32)
    x2 = np.asarray(inputs["x2"], f32)
    Wq, bq = np.asarray(inputs["Wq"], f32), np.asarray(inputs["bq"], f32)
    Wk, bk = np.asarray(inputs["Wk"], f32), np.asarray(inputs["bk"], f32)
    Wv, bv = np.asarray(inputs["Wv"], f32), np.asarray(inputs["bv"], f32)
    W1, W2 = np.asarray(inputs["W1"], f32), np.asarray(inputs["W2"], f32)
    g1, b1 = np.asarray(inputs["g1"], f32), np.asarray(inputs["b1"], f32)
    g2, b2 = np.asarray(inputs["g2"], f32), np.asarray(inputs["b2"], f32)
    Wa, ba = np.asarray(inputs["Wa"], f32), np.asarray(inputs["ba"], f32)
    bn_g, bn_b = np.asarray(inputs["bn_g"], f32), np.asarray(inputs["bn_b"], f32)
    bn_m, bn_v = np.asarray(inputs["bn_m"], f32), np.asarray(inputs["bn_v"], f32)
    thr = np.asarray(inputs["threshold"], f32)

    c = lambda a: np.ascontiguousarray(a, dtype=f32)
    cb = lambda a: np.ascontiguousarray(np.asarray(a, f32).astype(bf16))

    wkv = cb(np.concatenate([Wk.T, Wv.T], axis=1))           # [C, 2C]
    scale_bn = bn_g / np.sqrt(bn_v + BN_EPS)
    # fold BN affine AND the +x1 residual into the aug conv
    wa_f = cb((scale_bn[:, None] * Wa + np.eye(C, dtype=f32)).T)
    ba_f = (scale_bn * ba + (bn_b - bn_m * scale_bn) + b2)[:, None]
    W1a, W1b = W1[:, :C], W1[:, C:]
    w1a = cb(W1a.T)                                          # [C, 2C]
    w1b = cb((W1b * g1[None, :]).T)                          # [C, 2C]
    hb = c((W1b @ b1)[:, None])                              # [2C, 1]
    w2c = cb((W2 - W2.mean(axis=0, keepdims=True)).T)        # [2C, C] centered
    shared = {
        "wkv": wkv,
        "wq": cb(Wq.T),
        "bqn": c(-bq[:, None]), "bq1": c(bq[:, None] + 1.0),
        "wa": wa_f, "ba": c(ba_f),
        "w1a": w1a, "w1b": w1b, "hb": hb,
        "w2": w2c,
        "g2": c(g2[:, None]),
        "bkr": cb(bk[None, :]), "bvr": c(bv[None, :]),
        "thr": c(thr.reshape(1, 1)),
        "ident": np.ascontiguousarray(np.eye(128, dtype=f32).astype(bf16)),
    }
    in_maps = []
    for core in range(8):
        b_, half = core // 2, core % 2
        m = dict(shared)
        m["x1"] = np.ascontiguousarray(
            x1[b_][:, half * L:(half + 1) * L].astype(bf16))
        m["x2"] = np.ascontiguousarray(x2[b_].astype(bf16))
        in_maps.append(m)
    return in_maps


def _get_nc():
    if "nc" not in _CACHE:
        _CACHE["nc"] = _build()
    return _CACHE["nc"]


def kernel(**inputs) -> np.ndarray:
    nc = _get_nc()
    in_maps = _host_prep(inputs)
    res = run_bass_kernel_spmd(nc, in_maps, core_ids=list(range(8)),
                               trace=bool(int(os.environ.get("KBENCH_TRACE", "0"))))
    if os.environ.get("KBENCH_TIME_OUT"):
        with open(os.environ["KBENCH_TIME_OUT"], "w") as f:
            f.write(str(res.exec_time_ns))
    out = np.empty((B, C, N), np.float32)
    for core in range(8):
        b_, half = core // 2, core % 2
        out[b_][:, half * L:(half + 1) * L] = res.results[core]["out"]
    return out



# revision 2
# speedup vs baseline: 1.3436x; 1.3436x over previous
"""Trainium2 Bass kernel for nn_CrossAttention_59717225284223.

Full-input contract: kernel(**inputs) takes the complete [4,256,8192] tensors,
shards across 8 NeuronCores internally (core i -> batch i//2, N-half i%2; the
x2/KV side is recomputed per batch pair so no collectives are needed), and
returns the full [4,256,8192] float32 output.

v2: bf16 matmuls (FWL weight loads), bias/+1 terms folded into rank-1 matmul
updates, LN2 mean folded into centered W2, activations on the Scalar engine
(single act table: exp/ln/relu/copy/square), elementwise work spread across
Vector/GpSimd/Scalar so no single engine saturates.

v2.1: pin the activation-table chooser to natural_log_exp_and_others (kills
~42us of ACT_TABLE_LOAD thrash), threshold baked as an immediate (AP-scalar
is_gt was 2.2us/op), LN1 normalize entirely on Vector (GpSimd tensor_scalar
was 4.1us/op), LN1->MLP transposes moved to the DMA xbar (frees PE + copy
passes), kvb ones-columns memset hoisted out of the tile loop, deeper
phase-1 pipelining (more PSUM bufs).
"""

import os
import sys

import numpy as np

for _p in (
    "/root/.axon_site",
    "/root/.axon_site/_ro/trn_rl_repo",
    "/opt/trn_rl_repo",
):
    if os.path.isdir(_p) and _p not in sys.path:
        sys.path.append(_p)

import concourse.bass as bass  # noqa: E402
import concourse.tile as tile  # noqa: E402
from concourse import bacc, hw_specs, mybir  # noqa: E402
from concourse.bass_utils import run_bass_kernel_spmd  # noqa: E402

# All scalar activations used below (Relu/Exp/Ln/Copy/Square) live in the
# natural_log_exp_and_others table set; the default chooser greedily picks
# per-function sets and thrashes ACT_TABLE_LOAD (1.3us each) on every
# exp<->ln switch. Restrict it to the one set that has everything.
_orig_get_act_tables = hw_specs.get_activation_tables


def _single_set_act_tables(arch):
    t = _orig_get_act_tables(arch)
    return {k: (v if k == "natural_log_exp_and_others" else set())
            for k, v in t.items()}


bacc.get_activation_tables = _single_set_act_tables

F32 = mybir.dt.float32
BF16 = mybir.dt.bfloat16
OP = mybir.AluOpType
AF = mybir.ActivationFunctionType

B, C, N = 4, 256, 8192
H, D = 4, 64
L = N // 2          # positions per core
NT = N // 128       # x2-side 128-position tiles
NCH = L // 512      # 512-position chunks per core
LN_EPS = 1e-5
BN_EPS = 1e-5
ATTN_EPS = 1e-6

_CACHE = {}


def _build(thr_val: float):
    nc = bacc.Bacc(None, target_bir_lowering=False)

    x1 = nc.dram_tensor("x1", [C, L], BF16, kind="ExternalInput")
    x2 = nc.dram_tensor("x2", [C, N], BF16, kind="ExternalInput")
    wkv = nc.dram_tensor("wkv", [C, 2 * C], BF16, kind="ExternalInput")
    wq = nc.dram_tensor("wq", [C, C], BF16, kind="ExternalInput")
    wa = nc.dram_tensor("wa", [C, C], BF16, kind="ExternalInput")
    w1a = nc.dram_tensor("w1a", [C, 2 * C], BF16, kind="ExternalInput")
    w1b = nc.dram_tensor("w1b", [C, 2 * C], BF16, kind="ExternalInput")
    w2 = nc.dram_tensor("w2", [2 * C, C], BF16, kind="ExternalInput")
    bkr = nc.dram_tensor("bkr", [1, C], BF16, kind="ExternalInput")
    bvr = nc.dram_tensor("bvr", [1, C], F32, kind="ExternalInput")
    bqn = nc.dram_tensor("bqn", [C, 1], F32, kind="ExternalInput")
    bq1 = nc.dram_tensor("bq1", [C, 1], F32, kind="ExternalInput")
    ba = nc.dram_tensor("ba", [C, 1], F32, kind="ExternalInput")
    hbv = nc.dram_tensor("hb", [2 * C, 1], F32, kind="ExternalInput")
    g2 = nc.dram_tensor("g2", [C, 1], F32, kind="ExternalInput")
    out = nc.dram_tensor("out", [C, L], F32, kind="ExternalOutput")

    x1r = x1[:, :].rearrange("(t p) n -> p t n", p=128)
    x2r = x2[:, :].rearrange("(t p) n -> p t n", p=128)
    outr = out[:, :].rearrange("(t p) n -> p t n", p=128)

    with tile.TileContext(nc) as tc:
        with tc.tile_pool(name="consts", bufs=1) as consts, \
             tc.tile_pool(name="resident", bufs=1) as res:
            # ---- constants ----
            wkv_sb = consts.tile([128, 2, 2 * C], BF16)
            nc.sync.dma_start(out=wkv_sb, in_=wkv[:, :].rearrange(
                "(t p) o -> p t o", p=128))
            wq_sb = consts.tile([128, 2, C], BF16)
            nc.sync.dma_start(out=wq_sb, in_=wq[:, :].rearrange(
                "(t p) o -> p t o", p=128))
            wa_sb = consts.tile([128, 2, C], BF16)
            nc.sync.dma_start(out=wa_sb, in_=wa[:, :].rearrange(
                "(t p) o -> p t o", p=128))
            w1a_sb = consts.tile([128, 2, 2 * C], BF16)
            nc.sync.dma_start(out=w1a_sb, in_=w1a[:, :].rearrange(
                "(t p) o -> p t o", p=128))
            w1b_sb = consts.tile([128, 2, 2 * C], BF16)
            nc.sync.dma_start(out=w1b_sb, in_=w1b[:, :].rearrange(
                "(t p) o -> p t o", p=128))
            w2_sb = consts.tile([128, 4, C], BF16)
            nc.sync.dma_start(out=w2_sb, in_=w2[:, :].rearrange(
                "(t p) o -> p t o", p=128))
            bkr_sb = consts.tile([1, C], BF16)
            nc.sync.dma_start(out=bkr_sb, in_=bkr[:, :])
            bvr_sb = consts.tile([1, C], F32)
            nc.sync.dma_start(out=bvr_sb, in_=bvr[:, :])

            bqn_sb = consts.tile([128, 2], F32)
            bq1_sb = consts.tile([128, 2], F32)
            ba_sb = consts.tile([128, 2], F32)
            g2_sb = consts.tile([128, 2], F32)
            for t in range(2):
                sl = slice(t * 128, (t + 1) * 128)
                nc.sync.dma_start(out=bqn_sb[:, t:t + 1], in_=bqn[sl, :])
                nc.sync.dma_start(out=bq1_sb[:, t:t + 1], in_=bq1[sl, :])
                nc.sync.dma_start(out=ba_sb[:, t:t + 1], in_=ba[sl, :])
                nc.sync.dma_start(out=g2_sb[:, t:t + 1], in_=g2[sl, :])
            hb_sb = consts.tile([128, 4], F32)
            for t in range(4):
                nc.sync.dma_start(out=hb_sb[:, t:t + 1],
                                  in_=hbv[t * 128:(t + 1) * 128, :])
            ones_r = consts.tile([1, 128], BF16)
            nc.gpsimd.memset(ones_r, 1.0)
            ones_c = consts.tile([128, 1], BF16)
            nc.gpsimd.memset(ones_c, 1.0)
            lneps = consts.tile([128, 1], F32)
            nc.vector.memset(lneps, LN_EPS)
            eps11 = consts.tile([1, 1], F32)
            nc.vector.memset(eps11, LN_EPS)

            # ---- resident activations ----
            x1_sb = res.tile([128, 2, L], BF16)
            q_sb = res.tile([128, 2, L], BF16)
            msgn_sb = res.tile([128, 2, L], BF16)
            kvbd = res.tile([128, 2, 260], BF16)
            esum_sb = res.tile([128, 2], F32)
            # K/V staging tiles: allocated once so the ones columns
            # (512:514) are written a single time instead of per tile.
            kvb_t = [res.tile([128, 514], BF16, name=f"kvb{i}")
                     for i in range(6)]
            for kb in kvb_t:
                nc.gpsimd.memset(kb[:, 512:514], 1.0)

            # ================= phase 1: x2 side (full N) =================
            with tc.tile_pool(name="x2p", bufs=4) as x2p, \
                 tc.tile_pool(name="sc1", bufs=8) as sc1, \
                 tc.tile_pool(name="cps", bufs=5, space="PSUM") as cps, \
                 tc.tile_pool(name="kvps", bufs=1, space="PSUM") as kvps:
                kv_ps = [kvps.tile([128, 258], F32, name=f"kv_ps{m}",
                                   tag=f"kv{m}") for m in range(2)]
                for ch in range(N // 512):
                    x2t = x2p.tile([128, 2, 512], BF16)
                    nc.sync.dma_start(
                        out=x2t, in_=x2r[:, :, ch * 512:(ch + 1) * 512])
                    for s in range(4):
                        i = ch * 4 + s
                        cp = cps.tile([128, 2 * C], F32)
                        # bias rank-1 first: its LDWEIGHTS pulls ahead
                        nc.tensor.matmul(cp[:, 0:C], ones_r, bkr_sb,
                                         start=True, stop=False)
                        nc.tensor.matmul(cp, x2t[:, 0, s * 128:(s + 1) * 128],
                                         wkv_sb[:, 0, :], start=False,
                                         stop=False)
                        nc.tensor.matmul(cp, x2t[:, 1, s * 128:(s + 1) * 128],
                                         wkv_sb[:, 1, :], start=False,
                                         stop=True)
                        kvb = kvb_t[i % 6]
                        # elu(k)+1 = max(k+1, exp(min(k,0))); min via -relu(-k)
                        rn = sc1.tile([128, C], F32, name="rn", tag="rn")
                        nc.scalar.activation(out=rn, in_=cp[:, 0:C],
                                             func=AF.Relu, scale=-1.0)
                        ex = sc1.tile([128, C], F32, name="ex", tag="ex")
                        nc.scalar.activation(out=ex, in_=rn, func=AF.Exp,
                                             scale=-1.0)
                        nc.vector.scalar_tensor_tensor(
                            out=kvb[:, 0:C], in0=cp[:, 0:C], scalar=1.0,
                            in1=ex, op0=OP.add, op1=OP.max)
                        nc.vector.tensor_copy(out=kvb[:, C:2 * C],
                                              in_=cp[:, C:2 * C])
                        nc.tensor.matmul(kv_ps[0], kvb[:, 0:128],
                                         kvb[:, 256:514],
                                         start=(i == 0), stop=(i == NT - 1))
                        nc.tensor.matmul(kv_ps[1], kvb[:, 128:256],
                                         kvb[:, 256:514],
                                         start=(i == 0), stop=(i == NT - 1))

                # ---- KV fixup: V-bias rank-1 term ----
                # kvb K-half already holds K=elu(k)+1, so
                # KVfix = KVps + Ksum (x) bv, with Ksum = KVps[:, 256].
                bv_bc = sc1.tile([128, C], F32, name="bvbc", tag="bvbc")
                nc.gpsimd.partition_broadcast(bv_bc, bvr_sb)
                for t in range(2):
                    nc.vector.tensor_copy(out=esum_sb[:, t:t + 1],
                                          in_=kv_ps[t][:, 256:257])
                nc.gpsimd.memset(kvbd, 0.0)
                for t in range(2):
                    for hh in range(2):
                        h = t * 2 + hh
                        rsl = slice(hh * 64, hh * 64 + 64)
                        csl = slice(h * 64, h * 64 + 64)
                        nc.vector.scalar_tensor_tensor(
                            out=kvbd[rsl, t, csl], in0=bv_bc[rsl, csl],
                            scalar=esum_sb[rsl, t:t + 1],
                            in1=kv_ps[t][rsl, csl],
                            op0=OP.mult, op1=OP.add)
                        nc.gpsimd.tensor_copy(
                            out=kvbd[rsl, t, 256 + h:257 + h],
                            in_=esum_sb[rsl, t:t + 1])

            # ============ phase 2+3: q conv, msg, LN1 ============
            with tc.tile_pool(name="sc2", bufs=3) as sc2, \
                 tc.tile_pool(name="sc3", bufs=5) as sc3, \
                 tc.tile_pool(name="stat", bufs=4) as stat, \
                 tc.tile_pool(name="qaps", bufs=3, space="PSUM") as qaps, \
                 tc.tile_pool(name="msgps", bufs=1, space="PSUM") as msgps:
                for ch in range(NCH):
                    sl = slice(ch * 512, (ch + 1) * 512)
                    nc.sync.dma_start(out=x1_sb[:, :, sl], in_=x1r[:, :, sl])
                    for m in range(2):
                        qp = qaps.tile([128, 512], F32)
                        nc.tensor.matmul(qp, wq_sb[:, 0, m * 128:(m + 1) * 128],
                                         x1_sb[:, 0, sl], start=True, stop=False)
                        nc.tensor.matmul(qp, wq_sb[:, 1, m * 128:(m + 1) * 128],
                                         x1_sb[:, 1, sl], start=False, stop=True)
                        rq = sc2.tile([128, 512], F32, name="rq", tag="rq")
                        nc.scalar.activation(out=rq, in_=qp, func=AF.Relu,
                                             scale=-1.0, bias=bqn_sb[:, m:m + 1])
                        exq = sc2.tile([128, 512], F32, name="exq", tag="exq")
                        nc.scalar.activation(out=exq, in_=rq, func=AF.Exp,
                                             scale=-1.0)
                        nc.vector.scalar_tensor_tensor(
                            out=q_sb[:, m, sl], in0=qp,
                            scalar=bq1_sb[:, m:m + 1], in1=exq,
                            op0=OP.add, op1=OP.max)
                    zsc = stat.tile([128, 16], F32, name="zsc", tag="zsc")
                    mps = []
                    for s_ in range(4):
                        l0 = ch * 512 + s_ * 128
                        lsl = slice(l0, l0 + 128)
                        mp = msgps.tile([128, 260], F32, name=f"mp{s_}",
                                        tag=f"mp{s_}")
                        nc.tensor.matmul(mp, q_sb[:, 0, lsl], kvbd[:, 0, :],
                                         start=True, stop=False)
                        nc.tensor.matmul(mp, q_sb[:, 1, lsl], kvbd[:, 1, :],
                                         start=False, stop=True)
                        nc.vector.tensor_copy(out=zsc[:, s_ * 4:s_ * 4 + 4],
                                              in_=mp[:, 256:260])
                        mps.append(mp)
                    # sparse = scores * (scores > thr); z = 1/(sparse + eps)
                    mk = stat.tile([128, 16], F32, name="mk", tag="mk")
                    nc.vector.scalar_tensor_tensor(
                        out=mk, in0=zsc, scalar=thr_val, in1=zsc,
                        op0=OP.is_gt, op1=OP.mult)
                    nc.vector.tensor_scalar(
                        out=mk, in0=mk, scalar1=ATTN_EPS, scalar2=None,
                        op0=OP.add)
                    zt = stat.tile([128, 16], F32, name="zt", tag="zt")
                    nc.vector.reciprocal_approx_fast(out=zt, in_=mk)
                    mv8 = stat.tile([128, 4, 2], F32, name="mv8", tag="mv8")
                    mss = []
                    for s_ in range(4):
                        mp = mps[s_]
                        ms = sc3.tile([128, C], F32, name="ms", tag="ms")
                        zb = zt[:, s_ * 4:s_ * 4 + 4]
                        zb = bass.AP(tensor=zb.tensor, offset=zb.offset,
                                     ap=[list(zb.ap[0]), list(zb.ap[1]),
                                         [0, 64]])
                        nc.vector.tensor_tensor(
                            out=ms.rearrange("p (h d) -> p h d", h=4),
                            in0=mp[:, 0:256].rearrange("p (h d) -> p h d", h=4),
                            in1=zb, op=OP.mult)
                        st6 = stat.tile([128, 6], F32, name="st6", tag="st6")
                        nc.vector.bn_stats(out=st6, in_=ms)
                        nc.vector.bn_aggr(out=mv8[:, s_, :], in_=st6)
                        mss.append(ms)
                    # rz = 1/sqrt(var+eps) = exp(-0.5*ln(var+eps))
                    lnv = stat.tile([128, 4], F32, name="lnv", tag="lnv")
                    nc.scalar.activation(out=lnv, in_=mv8[:, :, 1],
                                         func=AF.Ln, bias=lneps)
                    rz = stat.tile([128, 4], F32, name="rz", tag="rz")
                    nc.scalar.activation(out=rz, in_=lnv, func=AF.Exp,
                                         scale=-0.5)
                    for s_ in range(4):
                        l0 = ch * 512 + s_ * 128
                        lsl = slice(l0, l0 + 128)
                        msn = sc3.tile([128, C], BF16, name="msn", tag="msn")
                        nc.vector.tensor_scalar(
                            out=msn, in0=mss[s_], scalar1=mv8[:, s_, 0:1],
                            scalar2=rz[:, s_:s_ + 1],
                            op0=OP.subtract, op1=OP.mult)
                        # LN1 -> MLP layout flip on the DMA xbar
                        for t in range(2):
                            nc.sync.dma_start_transpose(
                                out=msgn_sb[:, t, lsl],
                                in_=msn[:, t * 128:(t + 1) * 128])

            # ================= phase 4: MLP + LN2 + final =================
            with tc.tile_pool(name="hpool", bufs=5) as hpool, \
                 tc.tile_pool(name="sc4", bufs=4) as sc4, \
                 tc.tile_pool(name="bcp", bufs=2) as bcp, \
                 tc.tile_pool(name="outp", bufs=3) as outp, \
                 tc.tile_pool(name="hps", bufs=3, space="PSUM") as hps, \
                 tc.tile_pool(name="o2ps", bufs=1, space="PSUM") as o2ps, \
                 tc.tile_pool(name="stps", bufs=1, space="PSUM") as stps, \
                 tc.tile_pool(name="augps", bufs=2, space="PSUM") as augps:
                for ch in range(NCH):
                    sl = slice(ch * 512, (ch + 1) * 512)
                    hsb = []
                    for m in range(4):
                        mc = slice(m * 128, (m + 1) * 128)
                        hp = hps.tile([128, 512], F32)
                        nc.tensor.matmul(hp, w1a_sb[:, 0, mc], x1_sb[:, 0, sl],
                                         start=True, stop=False)
                        nc.tensor.matmul(hp, w1a_sb[:, 1, mc], x1_sb[:, 1, sl],
                                         start=False, stop=False)
                        nc.tensor.matmul(hp, w1b_sb[:, 0, mc],
                                         msgn_sb[:, 0, sl],
                                         start=False, stop=False)
                        nc.tensor.matmul(hp, w1b_sb[:, 1, mc],
                                         msgn_sb[:, 1, sl],
                                         start=False, stop=True)
                        ht = hpool.tile([128, 512], BF16)
                        if m % 2 == 0:
                            nc.scalar.activation(out=ht, in_=hp, func=AF.Relu,
                                                 bias=hb_sb[:, m:m + 1])
                        else:
                            nc.vector.tensor_scalar(
                                out=ht, in0=hp, scalar1=hb_sb[:, m:m + 1],
                                scalar2=0.0, op0=OP.add, op1=OP.max)
                        hsb.append(ht)
                    o2p = [o2ps.tile([128, 512], F32, name=f"o2p{m2}",
                                     tag=f"o2_{m2}") for m2 in range(2)]
                    for m2 in range(2):
                        mc2 = slice(m2 * 128, (m2 + 1) * 128)
                        for k in range(4):
                            nc.tensor.matmul(o2p[m2], w2_sb[:, k, mc2], hsb[k],
                                             start=(k == 0), stop=(k == 3))
                    # LN2: W2 pre-centered, so o2p is mean-free. var via
                    # sum of squares (Square on scalar, ones-matmul reduce).
                    sq = []
                    for m2 in range(2):
                        s_ = sc4.tile([128, 512], BF16, name=f"sq{m2}",
                                      tag=f"sq{m2}")
                        nc.scalar.activation(out=s_, in_=o2p[m2],
                                             func=AF.Square)
                        sq.append(s_)
                    ssq = stps.tile([1, 512], F32, tag="ssq")
                    nc.tensor.matmul(ssq, ones_c, sq[0], start=True, stop=False)
                    nc.tensor.matmul(ssq, ones_c, sq[1], start=False, stop=True)
                    # rstd = exp(-0.5*ln(ssq/C + eps))
                    lnv2 = sc4.tile([1, 512], F32, name="lnv2", tag="lnv2")
                    nc.scalar.activation(out=lnv2, in_=ssq, func=AF.Ln,
                                         scale=1.0 / C, bias=eps11)
                    rstd = sc4.tile([1, 512], F32, name="rstd", tag="rstd")
                    nc.scalar.activation(out=rstd, in_=lnv2, func=AF.Exp,
                                         scale=-0.5)
                    rstd_bc = bcp.tile([128, 512], F32, tag="rstd_bc")
                    nc.gpsimd.partition_broadcast(rstd_bc, rstd)
                    for m2 in range(2):
                        ap_ = augps.tile([128, 512], F32)
                        nc.tensor.matmul(ap_, wa_sb[:, 0, m2 * 128:(m2 + 1) * 128],
                                         x1_sb[:, 0, sl], start=True, stop=False)
                        nc.tensor.matmul(ap_, wa_sb[:, 1, m2 * 128:(m2 + 1) * 128],
                                         x1_sb[:, 1, sl], start=False, stop=True)
                        t1 = sc4.tile([128, 512], F32, name=f"t1{m2}",
                                      tag=f"t1{m2}")
                        nc.vector.scalar_tensor_tensor(
                            out=t1, in0=o2p[m2], scalar=g2_sb[:, m2:m2 + 1],
                            in1=rstd_bc, op0=OP.mult, op1=OP.mult)
                        ot = outp.tile([128, 512], F32)
                        nc.vector.scalar_tensor_tensor(
                            out=ot, in0=ap_, scalar=ba_sb[:, m2:m2 + 1], in1=t1,
                            op0=OP.add, op1=OP.add)
                        nc.sync.dma_start(out=outr[:, m2, sl], in_=ot)

    nc.compile()
    return nc


def _host_prep(inputs):
    """Fold BN/LN affine params into weights; build per-core input maps."""
    import ml_dtypes
    f32 = np.float32
    bf16 = ml_dtypes.bfloat16
    x1 = np.asarray(inputs["x1"], f32)
    x2 = np.asarray(inputs["x2"], f32)
    Wq, bq = np.asarray(inputs["Wq"], f32), np.asarray(inputs["bq"], f32)
    Wk, bk = np.asarray(inputs["Wk"], f32), np.asarray(inputs["bk"], f32)
    Wv, bv = np.asarray(inputs["Wv"], f32), np.asarray(inputs["bv"], f32)
    W1, W2 = np.asarray(inputs["W1"], f32), np.asarray(inputs["W2"], f32)
    g1, b1 = np.asarray(inputs["g1"], f32), np.asarray(inputs["b1"], f32)
    g2, b2 = np.asarray(inputs["g2"], f32), np.asarray(inputs["b2"], f32)
    Wa, ba = np.asarray(inputs["Wa"], f32), np.asarray(inputs["ba"], f32)
    bn_g, bn_b = np.asarray(inputs["bn_g"], f32), np.asarray(inputs["bn_b"], f32)
    bn_m, bn_v = np.asarray(inputs["bn_m"], f32), np.asarray(inputs["bn_v"], f32)

    c = lambda a: np.ascontiguousarray(a, dtype=f32)
    cb = lambda a: np.ascontiguousarray(np.asarray(a, f32).astype(bf16))

    wkv = cb(np.concatenate([Wk.T, Wv.T], axis=1))           # [C, 2C]
    scale_bn = bn_g / np.sqrt(bn_v + BN_EPS)
    # fold BN affine AND the +x1 residual into the aug conv
    wa_f = cb((scale_bn[:, None] * Wa + np.eye(C, dtype=f32)).T)
    ba_f = (scale_bn * ba + (bn_b - bn_m * scale_bn) + b2)[:, None]
    W1a, W1b = W1[:, :C], W1[:, C:]
    w1a = cb(W1a.T)                                          # [C, 2C]
    w1b = cb((W1b * g1[None, :]).T)                          # [C, 2C]
    hb = c((W1b @ b1)[:, None])                              # [2C, 1]
    w2c = cb((W2 - W2.mean(axis=0, keepdims=True)).T)        # [2C, C] centered
    shared = {
        "wkv": wkv,
        "wq": cb(Wq.T),
        "bqn": c(-bq[:, None]), "bq1": c(bq[:, None] + 1.0),
        "wa": wa_f, "ba": c(ba_f),
        "w1a": w1a, "w1b": w1b, "hb": hb,
        "w2": w2c,
        "g2": c(g2[:, None]),
        "bkr": cb(bk[None, :]), "bvr": c(bv[None, :]),
    }
    in_maps = []
    for core in range(8):
        b_, half = core // 2, core % 2
        m = dict(shared)
        m["x1"] = np.ascontiguousarray(
            x1[b_][:, half * L:(half + 1) * L].astype(bf16))
        m["x2"] = np.ascontiguousarray(x2[b_].astype(bf16))
        in_maps.append(m)
    return in_maps


def _get_nc(thr_val: float):
    key = ("nc", thr_val)
    if key not in _CACHE:
        _CACHE[key] = _build(thr_val)
    return _CACHE[key]


def kernel(**inputs) -> np.ndarray:
    thr_val = float(np.asarray(inputs["threshold"], np.float32).reshape(-1)[0])
    nc = _get_nc(thr_val)
    in_maps = _host_prep(inputs)
    res = run_bass_kernel_spmd(nc, in_maps, core_ids=list(range(8)),
                               trace=bool(int(os.environ.get("KBENCH_TRACE", "0"))))
    if os.environ.get("KBENCH_TIME_OUT"):
        with open(os.environ["KBENCH_TIME_OUT"], "w") as f:
            f.write(str(res.exec_time_ns))
    out = np.empty((B, C, N), np.float32)
    for core in range(8):
        b_, half = core // 2, core % 2
        out[b_][:, half * L:(half + 1) * L] = res.results[core]["out"]
    return out


# revision 8
# speedup vs baseline: 1.3514x; 1.0058x over previous
"""Trainium2 Bass kernel for nn_CrossAttention_59717225284223.

Full-input contract: kernel(**inputs) takes the complete [4,256,8192] tensors,
shards across 8 NeuronCores internally (core i -> batch i//2, N-half i%2; the
x2/KV side is recomputed per batch pair so no collectives are needed), and
returns the full [4,256,8192] float32 output.

v2.1: single activation-table set (kills ACT_TABLE_LOAD thrash), immediate
threshold, LN1 normalize on Vector, LN1->MLP transposes on the DMA xbar,
hoisted kvb ones columns, deeper phase-1 pipelining.

v2.2: fp8 (e4m3) DoubleRow matmuls for the x2-side K/V conv, the q conv and
the W1a half of the MLP (2 bf16 matmuls -> 1 fp8 matmul each). fp8 weights
are pre-scaled by 8 to stay in the normal range; the 8x cancels via free
scale/bias knobs downstream (activation scale, Z-denominator folding, LN2
renormalization), so no extra elementwise passes. Phase-1 bias rank-1
matmuls batched 4-at-a-time to share the ones-row stationary load.
"""

import os
import sys

import numpy as np

for _p in (
    "/root/.axon_site",
    "/root/.axon_site/_ro/trn_rl_repo",
    "/opt/trn_rl_repo",
):
    if os.path.isdir(_p) and _p not in sys.path:
        sys.path.append(_p)

import concourse.bass as bass  # noqa: E402
import concourse.tile as tile  # noqa: E402
from concourse import bacc, hw_specs, mybir  # noqa: E402
from concourse.bass_utils import run_bass_kernel_spmd  # noqa: E402

# All scalar activations used below (Relu/Exp/Ln/Copy/Square) live in the
# natural_log_exp_and_others table set; the default chooser greedily picks
# per-function sets and thrashes ACT_TABLE_LOAD (1.3us each) on every
# exp<->ln switch. Restrict it to the one set that has everything.
_orig_get_act_tables = hw_specs.get_activation_tables


def _single_set_act_tables(arch):
    t = _orig_get_act_tables(arch)
    return {k: (v if k == "natural_log_exp_and_others" else set())
            for k, v in t.items()}


bacc.get_activation_tables = _single_set_act_tables

F32 = mybir.dt.float32
BF16 = mybir.dt.bfloat16
FP8 = mybir.dt.float8e4
OP = mybir.AluOpType
AF = mybir.ActivationFunctionType
DR = mybir.MatmulPerfMode.DoubleRow

B, C, N = 4, 256, 8192
H, D = 4, 64
L = N // 2          # positions per core
NT = N // 128       # x2-side 128-position tiles
NCH = L // 512      # 512-position chunks per core
LN_EPS = 1e-5
BN_EPS = 1e-5
ATTN_EPS = 1e-6
WS = 8.0            # fp8 weight pre-scale
LN_WS = float(np.log(WS))

_CACHE = {}


def _build(thr_val: float):
    nc = bacc.Bacc(None, target_bir_lowering=False)

    x1 = nc.dram_tensor("x1", [C, L], BF16, kind="ExternalInput")
    x1p8 = nc.dram_tensor("x1p8", [128, 2, L], FP8, kind="ExternalInput")
    x2p8 = nc.dram_tensor("x2p8", [128, 2, N], FP8, kind="ExternalInput")
    wkv8 = nc.dram_tensor("wkv8", [128, 2, 2 * C], FP8, kind="ExternalInput")
    wq8 = nc.dram_tensor("wq8", [128, 2, C], FP8, kind="ExternalInput")
    w1a8 = nc.dram_tensor("w1a8", [128, 2, 2 * C], FP8, kind="ExternalInput")
    wa = nc.dram_tensor("wa", [C, C], BF16, kind="ExternalInput")
    w1b = nc.dram_tensor("w1b", [C, 2 * C], BF16, kind="ExternalInput")
    w2 = nc.dram_tensor("w2", [2 * C, C], BF16, kind="ExternalInput")
    bkr = nc.dram_tensor("bkr", [1, C], BF16, kind="ExternalInput")
    bvr = nc.dram_tensor("bvr", [1, C], F32, kind="ExternalInput")
    bqn = nc.dram_tensor("bqn", [C, 1], F32, kind="ExternalInput")
    qb8 = nc.dram_tensor("qb8", [C, 1], F32, kind="ExternalInput")
    ba = nc.dram_tensor("ba", [C, 1], F32, kind="ExternalInput")
    hbv = nc.dram_tensor("hb", [2 * C, 1], F32, kind="ExternalInput")
    g2 = nc.dram_tensor("g2", [C, 1], F32, kind="ExternalInput")
    out = nc.dram_tensor("out", [C, L], F32, kind="ExternalOutput")

    x1r = x1[:, :].rearrange("(t p) n -> p t n", p=128)
    outr = out[:, :].rearrange("(t p) n -> p t n", p=128)

    with tile.TileContext(nc) as tc:
        with tc.tile_pool(name="consts", bufs=1) as consts, \
             tc.tile_pool(name="resident", bufs=1) as res:
            # ---- constants ----
            wkv_sb = consts.tile([128, 2, 2 * C], FP8)
            nc.sync.dma_start(out=wkv_sb, in_=wkv8[:, :, :])
            wq_sb = consts.tile([128, 2, C], FP8)
            nc.sync.dma_start(out=wq_sb, in_=wq8[:, :, :])
            w1a_sb = consts.tile([128, 2, 2 * C], FP8)
            nc.sync.dma_start(out=w1a_sb, in_=w1a8[:, :, :])
            wa_sb = consts.tile([128, 2, C], BF16)
            nc.sync.dma_start(out=wa_sb, in_=wa[:, :].rearrange(
                "(t p) o -> p t o", p=128))
            w1b_sb = consts.tile([128, 2, 2 * C], BF16)
            nc.sync.dma_start(out=w1b_sb, in_=w1b[:, :].rearrange(
                "(t p) o -> p t o", p=128))
            w2_sb = consts.tile([128, 4, C], BF16)
            nc.sync.dma_start(out=w2_sb, in_=w2[:, :].rearrange(
                "(t p) o -> p t o", p=128))
            bkr_sb = consts.tile([1, C], BF16)
            nc.sync.dma_start(out=bkr_sb, in_=bkr[:, :])
            bvr_sb = consts.tile([1, C], F32)
            nc.sync.dma_start(out=bvr_sb, in_=bvr[:, :])

            bqn_sb = consts.tile([128, 2], F32)
            qb8_sb = consts.tile([128, 2], F32)
            ba_sb = consts.tile([128, 2], F32)
            g2_sb = consts.tile([128, 2], F32)
            for t in range(2):
                sl = slice(t * 128, (t + 1) * 128)
                nc.sync.dma_start(out=bqn_sb[:, t:t + 1], in_=bqn[sl, :])
                nc.sync.dma_start(out=qb8_sb[:, t:t + 1], in_=qb8[sl, :])
                nc.sync.dma_start(out=ba_sb[:, t:t + 1], in_=ba[sl, :])
                nc.sync.dma_start(out=g2_sb[:, t:t + 1], in_=g2[sl, :])
            hb_sb = consts.tile([128, 4], F32)
            for t in range(4):
                nc.sync.dma_start(out=hb_sb[:, t:t + 1],
                                  in_=hbv[t * 128:(t + 1) * 128, :])
            ones_r = consts.tile([1, 128], BF16)
            nc.gpsimd.memset(ones_r, 1.0)
            ones_c = consts.tile([128, 1], BF16)
            nc.gpsimd.memset(ones_c, 1.0)
            lneps = consts.tile([128, 1], F32)
            nc.vector.memset(lneps, LN_EPS)
            eps11 = consts.tile([1, 1], F32)
            nc.vector.memset(eps11, LN_EPS * WS * WS)
            one_b = consts.tile([128, 1], F32)
            nc.vector.memset(one_b, 1.0)
            lnws_b = consts.tile([128, 1], F32)
            nc.vector.memset(lnws_b, LN_WS)

            # ---- resident activations ----
            x1_sb = res.tile([128, 2, L], BF16)
            x1p_sb = res.tile([128, 2, L], FP8)
            nc.sync.dma_start(out=x1p_sb, in_=x1p8[:, :, :])
            q_sb = res.tile([128, 2, L], BF16)
            msgn_sb = res.tile([128, 2, L], BF16)
            kvbd = res.tile([128, 2, 260], BF16)
            esum_sb = res.tile([128, 2], F32)
            # K/V staging tiles: allocated once so the ones columns
            # (512:514) are written a single time instead of per tile.
            kvb_t = [res.tile([128, 514], BF16, name=f"kvb{i}")
                     for i in range(6)]
            for kb in kvb_t:
                nc.gpsimd.memset(kb[:, 512:514], 1.0)

            # ================= phase 1: x2 side (full N) =================
            # cp = WS*(k+1) for the K half (bias row = WS*(bk+1)) and WS*v
            # for the V half; the WS factor cancels downstream.
            with tc.tile_pool(name="x2p", bufs=3) as x2p, \
                 tc.tile_pool(name="sc1", bufs=8) as sc1, \
                 tc.tile_pool(name="cps", bufs=1, space="PSUM") as cps, \
                 tc.tile_pool(name="kvps", bufs=1, space="PSUM") as kvps:
                kv_ps = [kvps.tile([128, 258], F32, name=f"kv_ps{m}",
                                   tag=f"kv{m}") for m in range(2)]
                cp_t = [cps.tile([128, 2 * C], F32, name=f"cp{i}")
                        for i in range(6)]
                for ch in range(N // 512):
                    x2t = x2p.tile([128, 2, 512], FP8)
                    nc.sync.dma_start(
                        out=x2t, in_=x2p8[:, :, ch * 512:(ch + 1) * 512])
                    for s in range(4):
                        # bias rank-1 batched: shared ones stationary
                        nc.tensor.matmul(cp_t[(ch * 4 + s) % 6][:, 0:C],
                                         ones_r, bkr_sb,
                                         start=True, stop=False)
                    for s in range(4):
                        i = ch * 4 + s
                        cp = cp_t[i % 6]
                        nc.tensor.matmul(
                            cp, x2t[:, :, s * 128:(s + 1) * 128],
                            wkv_sb, start=False, stop=True, perf_mode=DR)
                        kvb = kvb_t[i % 6]
                        # WS*(elu(k)+1) = max(cp, WS*exp(min(k,0)))
                        rn = sc1.tile([128, C], F32, name="rn", tag="rn")
                        nc.scalar.activation(out=rn, in_=cp[:, 0:C],
                                             func=AF.Relu, scale=-1.0 / WS,
                                             bias=one_b)
                        ex = sc1.tile([128, C], F32, name="ex", tag="ex")
                        nc.scalar.activation(out=ex, in_=rn, func=AF.Exp,
                                             scale=-1.0, bias=lnws_b)
                        nc.vector.scalar_tensor_tensor(
                            out=kvb[:, 0:C], in0=cp[:, 0:C], scalar=0.0,
                            in1=ex, op0=OP.add, op1=OP.max)
                        nc.vector.tensor_copy(out=kvb[:, C:2 * C],
                                              in_=cp[:, C:2 * C])
                        nc.tensor.matmul(kv_ps[0], kvb[:, 0:128],
                                         kvb[:, 256:514],
                                         start=(i == 0), stop=(i == NT - 1))
                        nc.tensor.matmul(kv_ps[1], kvb[:, 128:256],
                                         kvb[:, 256:514],
                                         start=(i == 0), stop=(i == NT - 1))

                # ---- KV fixup: V-bias rank-1 term ----
                # kv entries are WS^2-scaled, esum columns WS-scaled;
                # bvr is WS*bv so the fixup lands at WS^2 as well.
                bv_bc = sc1.tile([128, C], F32, name="bvbc", tag="bvbc")
                nc.gpsimd.partition_broadcast(bv_bc, bvr_sb)
                for t in range(2):
                    nc.vector.tensor_copy(out=esum_sb[:, t:t + 1],
                                          in_=kv_ps[t][:, 256:257])
                nc.gpsimd.memset(kvbd, 0.0)
                for t in range(2):
                    for hh in range(2):
                        h = t * 2 + hh
                        rsl = slice(hh * 64, hh * 64 + 64)
                        csl = slice(h * 64, h * 64 + 64)
                        nc.vector.scalar_tensor_tensor(
                            out=kvbd[rsl, t, csl], in0=bv_bc[rsl, csl],
                            scalar=esum_sb[rsl, t:t + 1],
                            in1=kv_ps[t][rsl, csl],
                            op0=OP.mult, op1=OP.add)
                        nc.gpsimd.tensor_copy(
                            out=kvbd[rsl, t, 256 + h:257 + h],
                            in_=esum_sb[rsl, t:t + 1])

            # ============ phase 2+3: q conv, msg, LN1 ============
            # q_sb holds WS*(elu(q)+1); mp is WS^3-scaled for the KV part
            # and WS^2-scaled for the score columns -> fold into Z.
            with tc.tile_pool(name="sc2", bufs=3) as sc2, \
                 tc.tile_pool(name="sc3", bufs=5) as sc3, \
                 tc.tile_pool(name="stat", bufs=4) as stat, \
                 tc.tile_pool(name="qaps", bufs=3, space="PSUM") as qaps, \
                 tc.tile_pool(name="msgps", bufs=1, space="PSUM") as msgps:
                for ch in range(NCH):
                    sl = slice(ch * 512, (ch + 1) * 512)
                    nc.sync.dma_start(out=x1_sb[:, :, sl], in_=x1r[:, :, sl])
                    for m in range(2):
                        qp = qaps.tile([128, 512], F32)
                        nc.tensor.matmul(
                            qp, wq_sb[:, :, m * 128:(m + 1) * 128],
                            x1p_sb[:, :, sl], start=True, stop=True,
                            perf_mode=DR)
                        rq = sc2.tile([128, 512], F32, name="rq", tag="rq")
                        nc.scalar.activation(out=rq, in_=qp, func=AF.Relu,
                                             scale=-1.0 / WS,
                                             bias=bqn_sb[:, m:m + 1])
                        exq = sc2.tile([128, 512], F32, name="exq", tag="exq")
                        nc.scalar.activation(out=exq, in_=rq, func=AF.Exp,
                                             scale=-1.0, bias=lnws_b)
                        nc.vector.scalar_tensor_tensor(
                            out=q_sb[:, m, sl], in0=qp,
                            scalar=qb8_sb[:, m:m + 1], in1=exq,
                            op0=OP.add, op1=OP.max)
                    zsc = stat.tile([128, 16], F32, name="zsc", tag="zsc")
                    mps = []
                    for s_ in range(4):
                        l0 = ch * 512 + s_ * 128
                        lsl = slice(l0, l0 + 128)
                        mp = msgps.tile([128, 260], F32, name=f"mp{s_}",
                                        tag=f"mp{s_}")
                        nc.tensor.matmul(mp, q_sb[:, 0, lsl], kvbd[:, 0, :],
                                         start=True, stop=False)
                        nc.tensor.matmul(mp, q_sb[:, 1, lsl], kvbd[:, 1, :],
                                         start=False, stop=True)
                        nc.vector.tensor_copy(out=zsc[:, s_ * 4:s_ * 4 + 4],
                                              in_=mp[:, 256:260])
                        mps.append(mp)
                    # scores are WS^2-scaled: sparse = s*(s > WS^2*thr);
                    # Z' = 1/(WS^3*(sparse_true+eps)) cancels mp's WS^3.
                    mk = stat.tile([128, 16], F32, name="mk", tag="mk")
                    nc.vector.scalar_tensor_tensor(
                        out=mk, in0=zsc, scalar=thr_val * WS * WS, in1=zsc,
                        op0=OP.is_gt, op1=OP.mult)
                    nc.vector.tensor_scalar(
                        out=mk, in0=mk, scalar1=WS,
                        scalar2=ATTN_EPS * WS ** 3, op0=OP.mult, op1=OP.add)
                    zt = stat.tile([128, 16], F32, name="zt", tag="zt")
                    nc.vector.reciprocal_approx_fast(out=zt, in_=mk)
                    mv8 = stat.tile([128, 4, 2], F32, name="mv8", tag="mv8")
                    mss = []
                    for s_ in range(4):
                        mp = mps[s_]
                        ms = sc3.tile([128, C], F32, name="ms", tag="ms")
                        zb = zt[:, s_ * 4:s_ * 4 + 4]
                        zb = bass.AP(tensor=zb.tensor, offset=zb.offset,
                                     ap=[list(zb.ap[0]), list(zb.ap[1]),
                                         [0, 64]])
                        nc.vector.tensor_tensor(
                            out=ms.rearrange("p (h d) -> p h d", h=4),
                            in0=mp[:, 0:256].rearrange("p (h d) -> p h d", h=4),
                            in1=zb, op=OP.mult)
                        st6 = stat.tile([128, 6], F32, name="st6", tag="st6")
                        nc.vector.bn_stats(out=st6, in_=ms)
                        nc.vector.bn_aggr(out=mv8[:, s_, :], in_=st6)
                        mss.append(ms)
                    # rz = 1/sqrt(var+eps) = exp(-0.5*ln(var+eps))
                    lnv = stat.tile([128, 4], F32, name="lnv", tag="lnv")
                    nc.scalar.activation(out=lnv, in_=mv8[:, :, 1],
                                         func=AF.Ln, bias=lneps)
                    rz = stat.tile([128, 4], F32, name="rz", tag="rz")
                    nc.scalar.activation(out=rz, in_=lnv, func=AF.Exp,
                                         scale=-0.5)
                    # nmz = -mean*rz, so msn = ms*rz + nmz on the Scalar LUT
                    nmz = stat.tile([128, 4], F32, name="nmz", tag="nmz")
                    nc.vector.scalar_tensor_tensor(
                        out=nmz, in0=mv8[:, :, 0], scalar=-1.0, in1=rz,
                        op0=OP.mult, op1=OP.mult)
                    for s_ in range(4):
                        l0 = ch * 512 + s_ * 128
                        lsl = slice(l0, l0 + 128)
                        msn = sc3.tile([128, C], BF16, name="msn", tag="msn")
                        nc.scalar.activation(
                            out=msn, in_=mss[s_], func=AF.Identity,
                            scale=rz[:, s_:s_ + 1], bias=nmz[:, s_:s_ + 1])
                        # LN1 -> MLP layout flip on the DMA xbar
                        for t in range(2):
                            nc.sync.dma_start_transpose(
                                out=msgn_sb[:, t, lsl],
                                in_=msn[:, t * 128:(t + 1) * 128])

            # ================= phase 4: MLP + LN2 + final =================
            # h is WS-scaled (w1a8 = WS*W1a fp8, w1b/hb pre-scaled by WS);
            # LN2's rstd then comes out 1/WS-scaled and cancels exactly.
            with tc.tile_pool(name="hpool", bufs=5) as hpool, \
                 tc.tile_pool(name="sc4", bufs=4) as sc4, \
                 tc.tile_pool(name="bcp", bufs=2) as bcp, \
                 tc.tile_pool(name="outp", bufs=3) as outp, \
                 tc.tile_pool(name="hps", bufs=3, space="PSUM") as hps, \
                 tc.tile_pool(name="o2ps", bufs=1, space="PSUM") as o2ps, \
                 tc.tile_pool(name="stps", bufs=1, space="PSUM") as stps, \
                 tc.tile_pool(name="augps", bufs=2, space="PSUM") as augps:
                for ch in range(NCH):
                    sl = slice(ch * 512, (ch + 1) * 512)
                    hsb = []
                    for m in range(4):
                        mc = slice(m * 128, (m + 1) * 128)
                        hp = hps.tile([128, 512], F32)
                        nc.tensor.matmul(hp, w1a_sb[:, :, mc],
                                         x1p_sb[:, :, sl],
                                         start=True, stop=False, perf_mode=DR)
                        nc.tensor.matmul(hp, w1b_sb[:, 0, mc],
                                         msgn_sb[:, 0, sl],
                                         start=False, stop=False)
                        nc.tensor.matmul(hp, w1b_sb[:, 1, mc],
                                         msgn_sb[:, 1, sl],
                                         start=False, stop=True)
                        ht = hpool.tile([128, 512], BF16)
                        if m % 2 == 0:
                            nc.scalar.activation(out=ht, in_=hp, func=AF.Relu,
                                                 bias=hb_sb[:, m:m + 1])
                        else:
                            nc.vector.tensor_scalar(
                                out=ht, in0=hp, scalar1=hb_sb[:, m:m + 1],
                                scalar2=0.0, op0=OP.add, op1=OP.max)
                        hsb.append(ht)
                    o2p = [o2ps.tile([128, 512], F32, name=f"o2p{m2}",
                                     tag=f"o2_{m2}") for m2 in range(2)]
                    for m2 in range(2):
                        mc2 = slice(m2 * 128, (m2 + 1) * 128)
                        for k in range(4):
                            nc.tensor.matmul(o2p[m2], w2_sb[:, k, mc2], hsb[k],
                                             start=(k == 0), stop=(k == 3))
                    # LN2: W2 pre-centered, so o2p is mean-free. var via
                    # sum of squares (Square on scalar, ones-matmul reduce).
                    sq = []
                    for m2 in range(2):
                        s_ = sc4.tile([128, 512], BF16, name=f"sq{m2}",
                                      tag=f"sq{m2}")
                        nc.scalar.activation(out=s_, in_=o2p[m2],
                                             func=AF.Square)
                        sq.append(s_)
                    ssq = stps.tile([1, 512], F32, tag="ssq")
                    nc.tensor.matmul(ssq, ones_c, sq[0], start=True, stop=False)
                    nc.tensor.matmul(ssq, ones_c, sq[1], start=False, stop=True)
                    # rstd = exp(-0.5*ln(ssq/C + WS^2*eps)) = rstd_true/WS
                    lnv2 = sc4.tile([1, 512], F32, name="lnv2", tag="lnv2")
                    nc.scalar.activation(out=lnv2, in_=ssq, func=AF.Ln,
                                         scale=1.0 / C, bias=eps11)
                    rstd = sc4.tile([1, 512], F32, name="rstd", tag="rstd")
                    nc.scalar.activation(out=rstd, in_=lnv2, func=AF.Exp,
                                         scale=-0.5)
                    rstd_bc = bcp.tile([128, 512], F32, tag="rstd_bc")
                    nc.gpsimd.partition_broadcast(rstd_bc, rstd)
                    for m2 in range(2):
                        ap_ = augps.tile([128, 512], F32)
                        nc.tensor.matmul(ap_, wa_sb[:, 0, m2 * 128:(m2 + 1) * 128],
                                         x1_sb[:, 0, sl], start=True, stop=False)
                        nc.tensor.matmul(ap_, wa_sb[:, 1, m2 * 128:(m2 + 1) * 128],
                                         x1_sb[:, 1, sl], start=False, stop=True)
                        t1 = sc4.tile([128, 512], F32, name=f"t1{m2}",
                                      tag=f"t1{m2}")
                        nc.vector.scalar_tensor_tensor(
                            out=t1, in0=o2p[m2], scalar=g2_sb[:, m2:m2 + 1],
                            in1=rstd_bc, op0=OP.mult, op1=OP.mult)
                        ot = outp.tile([128, 512], F32)
                        nc.vector.scalar_tensor_tensor(
                            out=ot, in0=ap_, scalar=ba_sb[:, m2:m2 + 1], in1=t1,
                            op0=OP.add, op1=OP.add)
                        nc.sync.dma_start(out=outr[:, m2, sl], in_=ot)

    nc.compile()
    return nc


def _host_prep(inputs):
    """Fold BN/LN affine params into weights; build per-core input maps."""
    import ml_dtypes
    f32 = np.float32
    bf16 = ml_dtypes.bfloat16
    fp8 = ml_dtypes.float8_e4m3
    x1 = np.asarray(inputs["x1"], f32)
    x2 = np.asarray(inputs["x2"], f32)
    Wq, bq = np.asarray(inputs["Wq"], f32), np.asarray(inputs["bq"], f32)
    Wk, bk = np.asarray(inputs["Wk"], f32), np.asarray(inputs["bk"], f32)
    Wv, bv = np.asarray(inputs["Wv"], f32), np.asarray(inputs["bv"], f32)
    W1, W2 = np.asarray(inputs["W1"], f32), np.asarray(inputs["W2"], f32)
    g1, b1 = np.asarray(inputs["g1"], f32), np.asarray(inputs["b1"], f32)
    g2, b2 = np.asarray(inputs["g2"], f32), np.asarray(inputs["b2"], f32)
    Wa, ba = np.asarray(inputs["Wa"], f32), np.asarray(inputs["ba"], f32)
    bn_g, bn_b = np.asarray(inputs["bn_g"], f32), np.asarray(inputs["bn_b"], f32)
    bn_m, bn_v = np.asarray(inputs["bn_m"], f32), np.asarray(inputs["bn_v"], f32)

    c = lambda a: np.ascontiguousarray(a, dtype=f32)
    cb = lambda a: np.ascontiguousarray(np.asarray(a, f32).astype(bf16))
    c8 = lambda a: np.ascontiguousarray(np.asarray(a, f32).astype(fp8))

    # fp8 pair-packed weights, pre-scaled by WS ([128 pairs, 2, out])
    wkv8 = c8((np.float32(WS) * np.concatenate(
        [Wk.T, Wv.T], axis=1)).reshape(128, 2, 2 * C))
    wq8 = c8((np.float32(WS) * Wq.T).reshape(128, 2, C))
    W1a, W1b = W1[:, :C], W1[:, C:]
    w1a8 = c8((np.float32(WS) * W1a.T).reshape(128, 2, 2 * C))

    scale_bn = bn_g / np.sqrt(bn_v + BN_EPS)
    # fold BN affine AND the +x1 residual into the aug conv
    wa_f = cb((scale_bn[:, None] * Wa + np.eye(C, dtype=f32)).T)
    ba_f = (scale_bn * ba + (bn_b - bn_m * scale_bn) + b2)[:, None]
    w1b = cb(np.float32(WS) * (W1b * g1[None, :]).T)         # [C, 2C] WS-scaled
    hb = c(np.float32(WS) * (W1b @ b1)[:, None])             # [2C, 1] WS-scaled
    w2c = cb((W2 - W2.mean(axis=0, keepdims=True)).T)        # [2C, C] centered
    shared = {
        "wkv8": wkv8,
        "wq8": wq8,
        "w1a8": w1a8,
        "bqn": c(-bq[:, None]),
        "qb8": c(np.float32(WS) * (bq[:, None] + 1.0)),
        "wa": wa_f, "ba": c(ba_f),
        "w1b": w1b, "hb": hb,
        "w2": w2c,
        "g2": c(g2[:, None]),
        "bkr": cb(np.float32(WS) * (bk[None, :] + 1.0)),
        "bvr": c(np.float32(WS) * bv[None, :]),
    }
    in_maps = []
    for core in range(8):
        b_, half = core // 2, core % 2
        m = dict(shared)
        x1c = x1[b_][:, half * L:(half + 1) * L]
        m["x1"] = np.ascontiguousarray(x1c.astype(bf16))
        m["x1p8"] = c8(x1c.reshape(128, 2, L))
        m["x2p8"] = c8(x2[b_].reshape(128, 2, N))
        in_maps.append(m)
    return in_maps


def _get_nc(thr_val: float):
    key = ("nc", thr_val)
    if key not in _CACHE:
        _CACHE[key] = _build(thr_val)
    return _CACHE[key]


def kernel(**inputs) -> np.ndarray:
    thr_val = float(np.asarray(inputs["threshold"], np.float32).reshape(-1)[0])
    nc = _get_nc(thr_val)
    in_maps = _host_prep(inputs)
    res = run_bass_kernel_spmd(nc, in_maps, core_ids=list(range(8)),
                               trace=bool(int(os.environ.get("KBENCH_TRACE", "0"))))
    if os.environ.get("KBENCH_TIME_OUT"):
        with open(os.environ["KBENCH_TIME_OUT"], "w") as f:
            f.write(str(res.exec_time_ns))
    out = np.empty((B, C, N), np.float32)
    for core in range(8):
        b_, half = core // 2, core % 2
        out[b_][:, half * L:(half + 1) * L] = res.results[core]["out"]
    return out
